# revision 1
# baseline (speedup 1.0000x reference)
"""Trainium2 Bass kernel for CrossAttentionBlock (GN + 1x1-conv Q + text K/V
cross-attention + 1x1-conv proj + residual).

Sharding: data-parallel over batch — 16 batches / 8 cores = 2 batches per core.
Each core runs the identical SPMD program on its 2 batches.

Layout notes (per batch):
  x, xn, q, O, y are [C=512, HW=4096] channel-major, held as 4 row tiles of 128.
  Attention is computed in S^T form: S_T[t=77, hw] = K_h^T @ Q_h so the QK and
  AV matmuls stream N=512 moving operands.  Softmax runs over t (partitions):
  exp on ACT, denominator via an all-ones [77,64] stationary matmul that lands
  the per-head denominator replicated across the same 64 PSUM partitions as the
  head's AV output, so 1/denom (reciprocal_approx_fast) fuses into the
  PSUM->SBUF copy as a tensor_tensor multiply.
  The torch-view K/V reshape ([B,T,C] -> [B,8,64,77]) is a flat reinterpret, so
  K/V rows go through a DRAM scratch and come back as strided [64,77] slices.
"""

import math
import sys

sys.path.insert(0, "/opt/trn_rl_repo")

import numpy as np

import concourse.bass as bass
import concourse.mybir as mybir
import concourse.tile as tile_mod
from concourse.tile import TileContext

# ---------------------------------------------------------------------------
# Workaround: this walrus build rejects >1 sync wait on CTRL-type (Drain/Nop)
# instructions.  Replace TileContext's tail drain-with-N-waits by N single-wait
# nops followed by a wait-free drain.
# ---------------------------------------------------------------------------
from bass_rust import ScopedClock


def _split_drain_and_barrier(self, tick_clock, wait_clock):
    nc = self.nc
    coll = nc.sync.nop()
    wait_clock.add_sem_waits(coll.ins, ScopedClock({None: tick_clock.global_clock}))
    si = coll.ins.sync_info
    waits = list(si.on_wait) if si is not None else []
    coll.ins.sync_info = mybir.SyncInfo(on_wait=[], on_update=[])
    for w in waits:
        n = nc.sync.nop()
        n.ins.sync_info = mybir.SyncInfo(on_wait=[w], on_update=[])
    nc.sync.drain()
    nc.all_engine_barrier()
    assert self.sems is not None
    popped = nc._tile_sem_poison_stack.pop()
    assert popped is self._sem_poison
    nc.clear_and_free_semaphores(list(self.sems.allocated().values()))
    nc.all_engine_barrier()


tile_mod.TileContext._drain_and_barrier = _split_drain_and_barrier

F32 = mybir.dt.float32
F32R = mybir.dt.float32r
AF = mybir.ActivationFunctionType

B2 = 2          # batches per core
C = 512
HW = 4096
T = 77
FD = 768
HEADS = 8
D = 64
EPS = 1e-5
NCH = 8         # hw chunks per batch
CH = 512        # hw chunk size
NCT = 4         # channel tiles of 128
TPAD = 80       # padded per-head column stride for K tiles


def _split_excess_waits(nc, limit=1):
    """walrus in this env encodes at most one sync wait per instruction;
    hoist extras onto same-engine nops placed just before."""
    k = 0
    for f in nc.m.functions:
        for bb in f.blocks:
            insts = list(bb.instructions)
            out = []
            changed = False
            for inst in insts:
                si = getattr(inst, "sync_info", None)
                ow = list(si.on_wait) if si is not None else []
                if len(ow) > limit:
                    extra, keep = ow[:-limit], ow[-limit:]
                    for w in extra:
                        n = mybir.InstNoOp(name=f"wsplit-{k}", engine=inst.engine)
                        n.debug = inst.debug
                        n.bass_nofuse = False
                        k += 1
                        n.sync_info = mybir.SyncInfo(on_wait=[w], on_update=[])
                        out.append(n)
                    inst.sync_info = mybir.SyncInfo(
                        on_wait=keep, on_update=list(si.on_update))
                    changed = True
                out.append(inst)
            if changed:
                bb.instructions = out


def r(ap):
    """View an fp32 AP as float32r for fast PE matmuls."""
    return ap.bitcast(F32R)


def build_bass():
    nc = bass.Bass()

    x = nc.dram_tensor("x", [B2 * C, HW], F32, kind="ExternalInput")
    textT = nc.dram_tensor("textT", [B2 * FD, T], F32, kind="ExternalInput")
    qwT = nc.dram_tensor("qwT", [C, C], F32, kind="ExternalInput")
    pwT = nc.dram_tensor("pwT", [C, C], F32, kind="ExternalInput")
    kwT = nc.dram_tensor("kwT", [FD, C], F32, kind="ExternalInput")  # pre-scaled 1/8
    vwT = nc.dram_tensor("vwT", [FD, C], F32, kind="ExternalInput")
    qb = nc.dram_tensor("qb", [C], F32, kind="ExternalInput")
    kb = nc.dram_tensor("kb", [C], F32, kind="ExternalInput")  # pre-scaled 1/8
    vb = nc.dram_tensor("vb", [C], F32, kind="ExternalInput")
    pb = nc.dram_tensor("pb", [C], F32, kind="ExternalInput")
    gam = nc.dram_tensor("gam", [C], F32, kind="ExternalInput")
    bet = nc.dram_tensor("bet", [C], F32, kind="ExternalInput")
    gsel = nc.dram_tensor("gsel", [128, 8], F32, kind="ExternalInput")
    grepT = nc.dram_tensor("grepT", [8, 128], F32, kind="ExternalInput")
    ident = nc.dram_tensor("ident", [D, D], F32, kind="ExternalInput")
    ones77 = nc.dram_tensor("ones77", [T, D], F32, kind="ExternalInput")
    onesr77 = nc.dram_tensor("onesr77", [1, T], F32, kind="ExternalInput")
    onesr512 = nc.dram_tensor("onesr512", [1, CH], F32, kind="ExternalInput")
    y = nc.dram_tensor("y", [B2 * C, HW], F32, kind="ExternalOutput")

    kvf = nc.dram_tensor("kvf", [2, B2, T * C], F32)  # internal scratch

    from contextlib import ExitStack

    with nc.allow_low_precision(reason="f32r is 4-byte; rounding only"), \
         TileContext(nc) as tc, ExitStack() as stk:
        cp = stk.enter_context(tc.tile_pool(name="const", bufs=1))

        # --- resident constants / weights ---
        qw_sb = [cp.tile([128, C], F32, tag=f"qw{i}", name=f"qw{i}") for i in range(NCT)]
        pw_sb = [cp.tile([128, C], F32, tag=f"pw{i}", name=f"pw{i}") for i in range(NCT)]
        kw_sb = [cp.tile([128, C], F32, tag=f"kw{i}", name=f"kw{i}") for i in range(6)]
        vw_sb = [cp.tile([128, C], F32, tag=f"vw{i}", name=f"vw{i}") for i in range(6)]
        for i in range(NCT):
            nc.sync.dma_start(out=r(qw_sb[i][:]), in_=r(qwT[i * 128:(i + 1) * 128, :]))
            nc.sync.dma_start(out=r(pw_sb[i][:]), in_=r(pwT[i * 128:(i + 1) * 128, :]))
        for i in range(6):
            nc.sync.dma_start(out=r(kw_sb[i][:]), in_=r(kwT[i * 128:(i + 1) * 128, :]))
            nc.sync.dma_start(out=r(vw_sb[i][:]), in_=r(vwT[i * 128:(i + 1) * 128, :]))

        gsel_sb = cp.tile([128, 8], F32, tag="gsel")
        grepT_sb = cp.tile([8, 128], F32, tag="grepT")
        ident_sb = cp.tile([D, D], F32, tag="ident")
        ones77_sb = cp.tile([T, D], F32, tag="ones77")
        onesr77_sb = cp.tile([1, T], F32, tag="onesr77")
        onesr512_sb = cp.tile([1, CH], F32, tag="onesr512")
        nc.sync.dma_start(out=r(gsel_sb[:]), in_=r(gsel[:]))
        nc.sync.dma_start(out=r(grepT_sb[:]), in_=r(grepT[:]))
        nc.sync.dma_start(out=ident_sb[:], in_=ident[:])
        nc.sync.dma_start(out=r(ones77_sb[:]), in_=r(ones77[:]))
        nc.sync.dma_start(out=r(onesr77_sb[:]), in_=r(onesr77[:]))
        nc.sync.dma_start(out=r(onesr512_sb[:]), in_=r(onesr512[:]))

        # per-channel vectors as [128, 4] (col = channel tile)
        qb_sb = cp.tile([128, NCT], F32, tag="qb")
        gam_sb = cp.tile([128, NCT], F32, tag="gam")
        bet_sb = cp.tile([128, NCT], F32, tag="bet")
        for t_, src in ((qb_sb, qb), (gam_sb, gam), (bet_sb, bet)):
            nc.sync.dma_start(
                out=t_[:], in_=src.rearrange("(ci p) -> p ci", p=128))
        kb_row = cp.tile([1, C], F32, tag="kbrow")
        vb_row = cp.tile([1, C], F32, tag="vbrow")
        pb_row = cp.tile([1, C], F32, tag="pbrow")
        nc.sync.dma_start(out=r(kb_row[:]), in_=r(kb[None, :]))
        nc.sync.dma_start(out=r(vb_row[:]), in_=r(vb[None, :]))
        nc.sync.dma_start(out=r(pb_row[:]), in_=r(pb[None, :]))

        # K / V head tiles (filled in the KV phase)
        kh_sb = cp.tile([128, B2 * (HEADS // 2) * TPAD], F32, tag="khall")
        vt_sb = cp.tile([T, B2 * HEADS * D], F32, tag="vtall")
        # groupnorm per-channel alpha/beta per batch: cols b*8 + ci -> alpha,
        # b*8 + 4 + ci -> beta'
        albe_sb = cp.tile([128, B2 * 8], F32, tag="albe")

        # ------------------------------------------------------------------
        # Phase 1: K/V projections -> DRAM scratch -> per-head tiles
        # ------------------------------------------------------------------
        with tc.tile_pool(name="kvp", bufs=2) as kvp, \
             tc.tile_pool(name="kvpsum", bufs=2, space="PSUM") as kvps, \
             tc.tile_pool(name="vtpsum", bufs=2, space="PSUM") as vtps:
            for b in range(B2):
                tt = [kvp.tile([128, T], F32, tag=f"tt{fc}", name=f"tt{fc}") for fc in range(6)]
                for fc in range(6):
                    nc.sync.dma_start(
                        out=r(tt[fc][:]),
                        in_=r(textT[b * FD + fc * 128:b * FD + (fc + 1) * 128, :]))
                for j, (w_sb, b_row) in enumerate(
                        ((kw_sb, kb_row), (vw_sb, vb_row))):
                    ps = kvps.tile([T, C], F32, tag="kvps")
                    for fc in range(6):
                        nc.tensor.matmul(ps[:], r(tt[fc][:]), r(w_sb[fc][:]),
                                         start=(fc == 0), stop=False)
                    nc.tensor.matmul(ps[:], r(onesr77_sb[:]), r(b_row[:]),
                                     start=False, stop=True)
                    kv_out = kvp.tile([T, C], F32, tag="kvout")
                    nc.scalar.copy(out=kv_out[:], in_=ps[:])
                    nc.sync.dma_start(
                        out=kvf[j, b].rearrange("(t c) -> t c", c=C),
                        in_=kv_out[:])
                # per-head K tiles and V^T tiles
                for h in range(HEADS):
                    fl = kvf[0, b][h * (D * T):(h + 1) * (D * T)]
                    kcol = (b * (HEADS // 2) + h // 2) * TPAD
                    nc.sync.dma_start(
                        out=r(kh_sb[(h % 2) * 64:(h % 2) * 64 + D,
                                    kcol:kcol + T]),
                        in_=r(fl.rearrange("(d s) -> d s", s=T)))
                    flv = kvf[1, b][h * (D * T):(h + 1) * (D * T)]
                    vh = kvp.tile([D, T], F32, tag="vh")
                    nc.sync.dma_start(out=vh[:],
                                      in_=flv.rearrange("(d s) -> d s", s=T))
                    vtp = vtps.tile([T, D], F32, tag="vtp")
                    nc.tensor.transpose(vtp[:], vh[:], ident_sb[:])
                    nc.scalar.copy(
                        out=r(vt_sb[:, (b * HEADS + h) * D:
                                    (b * HEADS + h + 1) * D]),
                        in_=vtp[:])

        # ------------------------------------------------------------------
        # Phase 2: GroupNorm statistics for both batches
        # ------------------------------------------------------------------
        with tc.tile_pool(name="gnx", bufs=2) as gnx, \
             tc.tile_pool(name="gns", bufs=4) as gns, \
             tc.tile_pool(name="gnpsum", bufs=2, space="PSUM") as gnps:
            for b in range(B2):
                gst = gnps.tile([8, 2 * NCT], F32, tag="gst")
                for ci in range(NCT):
                    xt = gnx.tile([128, HW], F32, tag="xt")
                    nc.sync.dma_start(
                        out=xt[:],
                        in_=x[b * C + ci * 128:b * C + (ci + 1) * 128, :])
                    bno = gns.tile([128, NCH, 6], F32, tag="bno")
                    for ch in range(NCH):
                        nc.vector.bn_stats(bno[:, ch, :],
                                           xt[:, ch * CH:(ch + 1) * CH])
                    agg = gns.tile([128, 2], F32, tag="agg")
                    nc.vector.bn_aggr(agg[:], bno[:])
                    t2 = gns.tile([128, 2], F32, tag="t2")
                    nc.vector.tensor_copy(r(t2[:, 0:1]), agg[:, 0:1])
                    nc.vector.tensor_mul(r(t2[:, 1:2]), agg[:, 0:1], agg[:, 0:1])
                    nc.vector.tensor_add(r(t2[:, 1:2]), t2[:, 1:2], agg[:, 1:2])
                    nc.tensor.matmul(gst[:, 2 * ci:2 * ci + 2],
                                     r(gsel_sb[:]), r(t2[:]),
                                     start=(ci == 0), stop=(ci == NCT - 1))
                # gst cols (2ci, 2ci+1) = (sum mean_c, sum (var_c+mean_c^2))
                gs = gns.tile([8, 2 * NCT], F32, tag="gs")
                nc.scalar.mul(out=r(gs[:]), in_=gst[:], mul=1.0 / 16.0)
                # group var = E[x^2] - mu^2  (+eps), rs = 1/sqrt
                mg2 = gns.tile([8, NCT], F32, tag="mg2")
                nc.vector.tensor_mul(mg2[:], gs[:, 0:8:2], gs[:, 0:8:2])
                var = gns.tile([8, NCT], F32, tag="var")
                nc.vector.tensor_sub(var[:], gs[:, 1:8:2], mg2[:])
                ve = gns.tile([8, NCT], F32, tag="ve")
                nc.vector.tensor_scalar_add(out=ve[:], in0=var[:],
                                            scalar1=float(EPS))
                sq = gns.tile([8, NCT], F32, tag="sq")
                nc.scalar.activation(out=sq[:], in_=ve[:], func=AF.Sqrt)
                rs = gns.tile([8, NCT], F32, tag="rs")
                nc.vector.reciprocal(r(rs[:]), sq[:])
                # replicate groups -> channels: [128, 8] = [mu x4 | rs x4]
                rep = gnps.tile([128, 2 * NCT], F32, tag="rep")
                nc.tensor.matmul(rep[:, 0:NCT], r(grepT_sb[:]), r(gs[:, 0:8:2]),
                                 start=True, stop=False)
                nc.tensor.matmul(rep[:, NCT:2 * NCT], r(grepT_sb[:]), r(rs[:]),
                                 start=False, stop=True)
                mual = gns.tile([128, NCT], F32, tag="mual")
                nc.vector.tensor_mul(albe_sb[:, b * 8:b * 8 + 4],
                                     rep[:, NCT:2 * NCT], gam_sb[:])
                nc.vector.tensor_mul(mual[:], rep[:, 0:NCT],
                                     albe_sb[:, b * 8:b * 8 + 4])
                nc.vector.tensor_sub(albe_sb[:, b * 8 + 4:b * 8 + 8],
                                     bet_sb[:], mual[:])

        # ------------------------------------------------------------------
        # Phase 3: main chunk pipeline
        # ------------------------------------------------------------------
        with tc.tile_pool(name="xch", bufs=3) as xch, \
             tc.tile_pool(name="xnp", bufs=2) as xnp, \
             tc.tile_pool(name="qsb", bufs=2) as qsb, \
             tc.tile_pool(name="esb", bufs=4) as esb, \
             tc.tile_pool(name="rsb", bufs=3) as rsb, \
             tc.tile_pool(name="osb", bufs=2) as osb, \
             tc.tile_pool(name="ysb", bufs=4) as ysb, \
             tc.tile_pool(name="qps", bufs=2, space="PSUM") as qps, \
             tc.tile_pool(name="sps", bufs=2, space="PSUM") as sps, \
             tc.tile_pool(name="ops", bufs=2, space="PSUM") as ops, \
             tc.tile_pool(name="dps", bufs=2, space="PSUM") as dps:
            for b in range(B2):
                for n in range(NCH):
                    cs = slice(n * CH, (n + 1) * CH)
                    # GroupNorm apply -> xn chunk (per channel tile)
                    xc = []
                    xn = []
                    for ci in range(NCT):
                        xt = xch.tile([128, CH], F32, tag=f"xc{ci}")
                        nc.sync.dma_start(
                            out=xt[:],
                            in_=x[b * C + ci * 128:b * C + (ci + 1) * 128, cs])
                        xc.append(xt)
                        xnt = xnp.tile([128, CH], F32, tag=f"xn{ci}")
                        nc.scalar.activation(
                            out=r(xnt[:]), in_=xt[:], func=AF.Identity,
                            bias=albe_sb[:, b * 8 + 4 + ci:b * 8 + 5 + ci],
                            scale=albe_sb[:, b * 8 + ci:b * 8 + 1 + ci])
                        xn.append(xnt)
                    # Q projection (+bias on the PSUM->SBUF copy)
                    q = []
                    for oi in range(NCT):
                        qp = qps.tile([128, CH], F32, tag="qp")
                        for ci in range(NCT):
                            nc.tensor.matmul(
                                qp[:],
                                r(qw_sb[ci][:, oi * 128:(oi + 1) * 128]),
                                r(xn[ci][:]),
                                start=(ci == 0), stop=(ci == NCT - 1))
                        qt = qsb.tile([128, CH], F32, tag=f"q{oi}")
                        nc.scalar.activation(out=r(qt[:]), in_=qp[:],
                                             func=AF.Identity,
                                             bias=qb_sb[:, oi:oi + 1])
                        q.append(qt)
                    # attention per channel tile (= head pair)
                    o = []
                    for ci in range(NCT):
                        ot = osb.tile([128, CH], F32, tag=f"o{ci}", name=f"o{ci}")
                        for hh in range(2):
                            h = 2 * ci + hh
                            sp = sps.tile([T, CH], F32, tag="sp")
                            kcol = (b * (HEADS // 2) + ci) * TPAD
                            nc.tensor.matmul(
                                sp[:],
                                r(kh_sb[hh * 64:hh * 64 + D, kcol:kcol + T]),
                                r(q[ci][hh * 64:(hh + 1) * 64, :]),
                                start=True, stop=True)
                            es = esb.tile([T, CH], F32, tag="es")
                            nc.scalar.activation(out=r(es[:]), in_=sp[:],
                                                 func=AF.Exp)
                            oph = ops.tile([D, CH], F32, tag="op")
                            nc.tensor.matmul(
                                oph[:],
                                r(vt_sb[:, (b * HEADS + h) * D:
                                        (b * HEADS + h + 1) * D]),
                                r(es[:]), start=True, stop=True)
                            dph = dps.tile([D, CH], F32, tag="dp")
                            nc.tensor.matmul(
                                dph[:], r(ones77_sb[:]), r(es[:]),
                                start=True, stop=True)
                            rt = rsb.tile([D, CH], F32, tag="rt")
                            nc.vector.reciprocal(rt[:], dph[:])
                            nc.vector.tensor_mul(
                                r(ot[hh * 64:(hh + 1) * 64, :]), oph[:], rt[:])
                        o.append(ot)
                    # output projection + pb + residual
                    for oi in range(NCT):
                        pp = qps.tile([128, CH], F32, tag="qp")
                        for ci in range(NCT):
                            nc.tensor.matmul(
                                pp[:],
                                r(pw_sb[ci][:, oi * 128:(oi + 1) * 128]),
                                r(o[ci][:]),
                                start=(ci == 0), stop=False)
                        nc.tensor.matmul(
                            pp[:], r(pb_row[:, oi * 128:(oi + 1) * 128]),
                            r(onesr512_sb[:]), start=False, stop=True)
                        yt = ysb.tile([128, CH], F32, tag="yt")
                        nc.vector.tensor_add(yt[:], pp[:], xc[oi][:])
                        nc.sync.dma_start(
                            out=y[b * C + oi * 128:b * C + (oi + 1) * 128, cs],
                            in_=yt[:])
    _split_excess_waits(nc)
    return nc


_CACHE = {}


def _get_nc():
    if "nc" not in _CACHE:
        _CACHE["nc"] = build_bass()
    return _CACHE["nc"]


def kernel(x, text_emb, gn_gamma, gn_beta, q_w, q_b, k_w, k_b, v_w, v_b,
           proj_w, proj_b, _trace=False, _trace_kwargs=None):
    from concourse.bass_utils import run_bass_kernel_spmd

    x = np.asarray(x, dtype=np.float32)
    text_emb = np.asarray(text_emb, dtype=np.float32)
    B = x.shape[0]
    n_cores = 8
    per = B // n_cores
    assert per * n_cores == B

    scale = 1.0 / math.sqrt(D)
    qwT = np.ascontiguousarray(np.asarray(q_w, np.float32).T)
    pwT = np.ascontiguousarray(np.asarray(proj_w, np.float32).T)
    kwT = np.ascontiguousarray(np.asarray(k_w, np.float32).T * scale)
    vwT = np.ascontiguousarray(np.asarray(v_w, np.float32).T)
    kb_s = np.asarray(k_b, np.float32) * scale

    gsel = np.zeros((128, 8), np.float32)
    for p in range(128):
        gsel[p, p // 16] = 1.0
    grepT = np.ascontiguousarray(gsel.T)
    ident = np.eye(D, dtype=np.float32)

    shared = dict(
        qwT=qwT, pwT=pwT, kwT=kwT, vwT=vwT,
        qb=np.asarray(q_b, np.float32), kb=kb_s,
        vb=np.asarray(v_b, np.float32), pb=np.asarray(proj_b, np.float32),
        gam=np.asarray(gn_gamma, np.float32), bet=np.asarray(gn_beta, np.float32),
        gsel=gsel, grepT=grepT, ident=ident,
        ones77=np.ones((T, D), np.float32),
        onesr77=np.ones((1, T), np.float32),
        onesr512=np.ones((1, CH), np.float32),
    )

    in_maps = []
    for c in range(n_cores):
        xc = np.ascontiguousarray(
            x[c * per:(c + 1) * per].reshape(per * C, HW))
        tc_ = np.ascontiguousarray(
            text_emb[c * per:(c + 1) * per].transpose(0, 2, 1).reshape(
                per * FD, T))
        in_maps.append(dict(shared, x=xc, textT=tc_))

    nc = _get_nc()
    kwargs = {}
    if _trace:
        kwargs["trace"] = True
        kwargs["trace_cores"] = [0]
        if _trace_kwargs:
            kwargs.update(_trace_kwargs)
    res = run_bass_kernel_spmd(nc, in_maps, core_ids=list(range(n_cores)),
                               **kwargs)
    out = np.empty((B, C, 64, 64), np.float32)
    for c in range(n_cores):
        out[c * per:(c + 1) * per] = res.results[c]["y"].reshape(per, C, 64, 64)
    if _trace:
        _CACHE["last_results"] = res
    return out

